# revision 14
# baseline (speedup 1.0000x reference)
"""MoE top-1 routing layer on 8 Trainium2 NeuronCores (expert-parallel).

Math: out[t] = (x[t] @ W[e] + b[e]) @ OW + ob   with e = argmax(x[t] @ GW + gb).

Decomposition used here:
  out[t] = (x[t] @ W[e]) @ OW + bias2[e],   bias2[e] = b[e] @ OW + ob
so the device only runs two chained matmuls per core; the per-expert bias
constant is added by the host during unshard.

Sharding: expert-parallel. Host computes the gate (fp64 -> exact argmax),
sorts tokens by expert, pads each expert's token set to capacity C, and
ships core k: xT (gathered tokens, transposed), W[k], OW. Each core returns
its C token outputs; host scatters rows back and adds bias2. Tokens beyond
capacity (never for balanced routing) fall back to a host matmul.

Device inputs are host-packed into SBUF-stripe-major layouts so each DMA is
a single trigger with multi-KB contiguous descriptors:
  xt{i}: [128, D/128, BLK_i]  (one per token block)
  w:     [H/128, 128, D/128, 128]  (h-ptile major)
  ow:    [128, H/128, O]
The first token block is small (256) so the first matmul is gated on only
~0.8 MB of DMA.
"""

import numpy as np
from contextlib import ExitStack

B, S, D, E, H, O = 4, 2048, 1024, 8, 2048, 1024
T = B * S
C = 1152          # per-expert token capacity (multiple of 128)
P = 128
KO_D = D // P     # 8
KO_H = H // P     # 16

# "bf16": all matmul operands bf16 (fp32 PSUM accumulation) — fastest, rel
#         err ~4e-3. "f32r": fp32-width storage with reduced-precision
#         multiply — rel err ~2e-4, ~20% slower.
MM_DT = "bf16"

BLOCKS = [256, 512, 384]          # token blocks (sum == C)
assert sum(BLOCKS) == C


def _legalize_waits(nc):
    """This container's walrus accepts 1 sem wait per instruction (2 for
    EventSemaphore); Tile's tail drain can carry more. Split the excess
    onto preceding same-engine NoOps."""
    from concourse import mybir

    uid = 0
    for f in nc.m.functions:
        for b in f.blocks:
            insts = b.instructions
            out = []
            changed = False
            for ins in insts:
                si = ins.sync_info
                waits = list(si.on_wait) if si is not None else []
                limit = 2 if str(ins.opcode) == "EventSemaphore" else 1
                if len(waits) > limit:
                    extra, keep = waits[:-limit], waits[-limit:]
                    for w in extra:
                        uid += 1
                        out.append(
                            mybir.InstNoOp(
                                name=f"waitsplit-{uid}",
                                engine=ins.engine,
                                sync_info=mybir.SyncInfo(on_wait=[w], on_update=[]),
                                bass_nofuse=True,
                            )
                        )
                    si.on_wait = keep
                    changed = True
                out.append(ins)
            if changed:
                insts.clear()
                insts.extend(out)


def _patch_tail_barrier(tile_mod):
    """Tile's kernel tail is drain -> barrier -> sem-reset -> barrier.
    The second all-engine barrier only orders the sem-reset against program
    end, which the per-engine stream end already guarantees; drop it."""
    if getattr(tile_mod.TileContext, "_moe_tail_patched", False):
        return
    from concourse.vector_clock import ScopedClock

    def _drain_and_barrier(self, tick_clock, wait_clock):
        drain_inst = self.nc.sync.drain()
        wait_clock.add_sem_waits(
            drain_inst.ins, ScopedClock({None: tick_clock.global_clock})
        )
        self.nc.all_engine_barrier()
        popped = self.nc._tile_sem_poison_stack.pop()
        assert popped is self._sem_poison
        self.nc.clear_and_free_semaphores(list(self.sems.allocated().values()))

    tile_mod.TileContext._drain_and_barrier = _drain_and_barrier
    tile_mod.TileContext._moe_tail_patched = True


def _emit(nc, tile, mm_dt, f32):
    """Two-matmul chain; x, W, OW all SBUF-resident."""
    xts = [
        nc.dram_tensor(f"xt{i}", [P, KO_D, bw], mm_dt, kind="ExternalInput")
        for i, bw in enumerate(BLOCKS)
    ]
    w = nc.dram_tensor("w", [KO_H, P, KO_D, P], mm_dt, kind="ExternalInput")
    ow = nc.dram_tensor("ow", [P, KO_H, O], mm_dt, kind="ExternalInput")
    out = nc.dram_tensor("out", [C, O], f32, kind="ExternalOutput")

    with tile.TileContext(nc) as tc:
        with ExitStack() as ctx:
            x_pool = ctx.enter_context(tc.tile_pool(name="x", bufs=1))
            w_pool = ctx.enter_context(tc.tile_pool(name="w", bufs=1))
            ow_pool = ctx.enter_context(tc.tile_pool(name="ow", bufs=1))
            h1_pool = ctx.enter_context(tc.tile_pool(name="h1", bufs=1))
            st_pool = ctx.enter_context(tc.tile_pool(name="st", bufs=2))
            ps_pool = ctx.enter_context(
                tc.tile_pool(name="ps", bufs=8, space="PSUM")
            )

            w_sb = w_pool.tile([P, KO_H, KO_D, P], mm_dt)
            ow_sb = ow_pool.tile([P, KO_H, O], mm_dt)
            x_sbs = [
                x_pool.tile([P, KO_D, bw], mm_dt, name=f"x{i}")
                for i, bw in enumerate(BLOCKS)
            ]
            h1_sbs = [
                h1_pool.tile([P, KO_H, bw], mm_dt, name=f"h1_{i}")
                for i, bw in enumerate(BLOCKS)
            ]

            # demand-ordered loads, one trigger each
            nc.sync.dma_start(x_sbs[0][:], xts[0][:])
            for h in range(8):
                nc.sync.dma_start(w_sb[:, h], w[h])
            nc.sync.dma_start(x_sbs[1][:], xts[1][:])
            for h in range(8, KO_H):
                nc.sync.dma_start(w_sb[:, h], w[h])
            nc.sync.dma_start(x_sbs[2][:], xts[2][:])
            nc.sync.dma_start(ow_sb[:], ow[:])

            def mm1_block(cs):
                bw = BLOCKS[cs]
                for h in range(KO_H):
                    ps = ps_pool.tile([P, 512], f32, name="ps")[:, :bw]
                    for k in range(KO_D):
                        nc.tensor.matmul(
                            ps,
                            w_sb[:, h, k],
                            x_sbs[cs][:, k],
                            start=(k == 0),
                            stop=(k == KO_D - 1),
                        )
                    nc.vector.tensor_copy(h1_sbs[cs][:, h], ps)

            def mm2_block(cs):
                c0 = sum(BLOCKS[:cs])
                for t in range(BLOCKS[cs] // P):
                    st = st_pool.tile([P, O], f32)
                    r0 = c0 + t * P
                    for o2 in range(O // 512):
                        ps2 = ps_pool.tile([P, 512], f32, name="ps")
                        for kh in range(KO_H):
                            nc.tensor.matmul(
                                ps2,
                                h1_sbs[cs][:, kh, t * P : (t + 1) * P],
                                ow_sb[:, kh, o2 * 512 : (o2 + 1) * 512],
                                start=(kh == 0),
                                stop=(kh == KO_H - 1),
                            )
                        nc.vector.tensor_copy(st[:, o2 * 512 : (o2 + 1) * 512], ps2)
                        nc.sync.dma_start(
                            out[r0 : r0 + P, o2 * 512 : (o2 + 1) * 512],
                            st[:, o2 * 512 : (o2 + 1) * 512],
                        )

            for cs in range(len(BLOCKS)):
                mm1_block(cs)
            for cs in range(len(BLOCKS)):
                mm2_block(cs)
    return nc


def _patch_walrus_policy():
    """Compile with walrus --policy=2 (heuristics post-scheduler): measured
    ~1.5us faster than the default --policy=0 on this kernel."""
    import concourse.bass_utils as bu

    if getattr(bu, "_moe_policy_patched", False):
        return
    orig = bu.run_command

    def _rc(argv, **kw):
        if argv and "walrus_driver" in str(argv[0]):
            argv = ["--policy=2" if a == "--policy=0" else a for a in argv]
        return orig(argv, **kw)

    bu.run_command = _rc
    bu._moe_policy_patched = True


def _build_nc():
    import concourse.bass as bass
    import concourse.tile as tile
    from concourse import mybir

    _patch_tail_barrier(tile)
    _patch_walrus_policy()
    f32 = mybir.dt.float32
    mm_dt = mybir.dt.bfloat16 if MM_DT == "bf16" else mybir.dt.float32r
    nc = bass.Bass()
    _emit(nc, tile, mm_dt, f32)
    _legalize_waits(nc)
    return nc


_NC_CACHE = {}


def kernel(x, gate_w, gate_b, expert_w, expert_b, out_w, out_b):
    import os

    # The device path runs through the axon PJRT plugin; make sure a
    # harness-pinned JAX_PLATFORMS=cpu doesn't exclude it.
    plats = os.environ.get("JAX_PLATFORMS")
    if plats and "axon" not in plats:
        os.environ["JAX_PLATFORMS"] = plats + ",axon"

    from concourse.bass_utils import run_bass_kernel_spmd

    x = np.asarray(x, dtype=np.float32)
    gate_w = np.asarray(gate_w, dtype=np.float32)
    gate_b = np.asarray(gate_b, dtype=np.float32)
    expert_w = np.asarray(expert_w, dtype=np.float32)
    expert_b = np.asarray(expert_b, dtype=np.float32)
    out_w = np.asarray(out_w, dtype=np.float32)
    out_b = np.asarray(out_b, dtype=np.float32)

    xt = x.reshape(T, D)
    # Gate on host in fp64: argmax matches the fp32 reference exactly
    # (min top-2 logit gap is ~1e-5, fp64 error ~1e-12).
    logits = xt.astype(np.float64) @ gate_w.astype(np.float64) + gate_b.astype(
        np.float64
    )
    idx = np.argmax(logits, axis=1)

    if MM_DT == "bf16":
        import ml_dtypes

        mm_np = ml_dtypes.bfloat16
    else:
        mm_np = np.float32

    # w packed [KO_H, P, KO_D, P]: w[h, p, k, j] = W[k*128+p, h*128+j]
    def pack_w(W):
        return np.ascontiguousarray(
            W.astype(mm_np).reshape(KO_D, P, KO_H, P).transpose(2, 1, 0, 3)
        )

    # ow packed [P, KO_H, O]: ow[p, k, j] = OW[k*128+p, j]
    ow_dev = np.ascontiguousarray(
        out_w.astype(mm_np).reshape(KO_H, P, O).transpose(1, 0, 2)
    )

    tok_of_expert = [np.nonzero(idx == e)[0] for e in range(E)]
    in_maps = []
    kept = []
    overflow = []
    for e in range(E):
        toks = tok_of_expert[e]
        if len(toks) > C:
            overflow.append((e, toks[C:]))
            toks = toks[:C]
        kept.append(toks)
        xpad = np.zeros((D, C), dtype=mm_np)
        xpad[:, : len(toks)] = xt[toks].T.astype(mm_np)
        # xt{i}[p, k, j] = xpad[k*128+p, c0+j]
        xk = xpad.reshape(KO_D, P, C)
        im = {"w": pack_w(expert_w[e]), "ow": ow_dev}
        c0 = 0
        for i, bw in enumerate(BLOCKS):
            im[f"xt{i}"] = np.ascontiguousarray(
                xk[:, :, c0 : c0 + bw].transpose(1, 0, 2)
            )
            c0 += bw
        in_maps.append(im)

    if "nc" not in _NC_CACHE:
        _NC_CACHE["nc"] = _build_nc()
    nc = _NC_CACHE["nc"]

    res = run_bass_kernel_spmd(nc, in_maps, list(range(E)))

    bias2 = (
        expert_b.astype(np.float64) @ out_w.astype(np.float64)
        + out_b.astype(np.float64)
    ).astype(np.float32)  # [E, O]

    out = np.empty((T, O), dtype=np.float32)
    for e in range(E):
        toks = kept[e]
        out[toks] = res.results[e]["out"][: len(toks)] + bias2[e]
    for e, toks in overflow:
        h1 = xt[toks] @ expert_w[e]
        out[toks] = h1 @ out_w + bias2[e]
    return out.reshape(B, S, O)
